# Initial kernel scaffold
#
"""Multi-head attention (B=4, S=1024, D=1024, H=16) on 8 Trainium2 NeuronCores.

Sharding (Megatron-style, per the hint): core c handles batch b = c//2 and
head-group hg = c%2 (8 heads = 512 channels of the QKV projections). Each
core computes its 8 heads' attention plus the partial output projection
y_part = attn_local @ Wo[:, ch].T; the host sums the two partials per batch
and adds bo.

Device kernel (bf16 matmuls, fp32 PSUM):
  - projections produce qhT/khT in [c, s] layout and vh in [s, c] layout
    (with a ones column appended per head for the softmax denominator)
  - scoresT[sk, sq] per head via khT-stationary matmuls, two heads row-packed
    into PE row-groups 0-63/64-127 so they run concurrently
  - additive mask (0 / -1e9, precomputed on host) accumulated into the score
    PSUM via an identity-stationary matmul
  - exp on ScalarE -> bf16 expT tiles; AV via vh-stationary matmuls gives
    attnT[c, sq] with the denominator in row 64; normalization via DVE
    reciprocal of the denominator row, a K=1 fp32 matmul that broadcasts it
    across partitions, and a DVE multiply
  - block plan computed from the actual mask contents skips fully-masked
    [sk-chunk x sq-range] blocks (causal mask -> ~2x attention-phase savings);
    an all-ones mask degenerates to the dense plan, still correct
"""

import math

import numpy as np
import ml_dtypes

import concourse.bass as bass
import concourse.mybir as mybir
import concourse.tile as tile
from bass_rust import ScopedClock, SyncInfo

BF16 = ml_dtypes.bfloat16
F32 = mybir.dt.float32
BF = mybir.dt.bfloat16

P = 128
B, S, D, H = 4, 1024, 1024, 16
DK = D // H           # 64
HLOC = H // 2         # 8 heads per core
C = HLOC * DK         # 512 local channels
NSK = S // P          # 8 sk chunks of 128
NSJ = 2               # sq chunks of 512
NEG = -1.0e9


# ----------------------------------------------------------------------------
# Walrus in this container rejects Drain instructions carrying more than one
# sync-wait command, and the leader/follower all-engine barrier. Override the
# TileContext exit sequence: split the tail drain's waits one-per-Drain and
# use the sem-only (EVSEM) barrier.
# ----------------------------------------------------------------------------
class PatchedTileContext(tile.TileContext):
    def _drain_and_barrier(self, tick_clock, wait_clock):
        nc = self.nc
        probe = nc.sync.drain()
        wait_clock.add_sem_waits(
            probe.ins, ScopedClock({None: tick_clock.global_clock})
        )
        si = probe.ins.sync_info
        waits = list(si.on_wait) if si is not None else []
        if len(waits) > 1:
            probe.ins.sync_info = SyncInfo(on_wait=waits[:1], on_update=[])
            for w in waits[1:]:
                extra = nc.sync.drain()
                extra.ins.sync_info = SyncInfo(on_wait=[w], on_update=[])
        nc.all_engine_barrier(sem_only=True)
        assert self.sems is not None
        popped = nc._tile_sem_poison_stack.pop()
        assert popped is self._sem_poison
        nc.clear_and_free_semaphores(list(self.sems.allocated().values()))
        nc.all_engine_barrier(sem_only=True)


def _install_wait_split(nc, max_waits: int = 1):
    """Walrus in this container rejects instructions carrying more than one
    sync-wait command. Post-process the serialized BIR: hoist excess on_wait
    entries of any instruction onto EventSemaphore instructions inserted just
    before it on the same engine (sequencers execute in order, so this is
    equivalent)."""
    import json as _json

    orig = nc.to_json_bytes
    counter = [0]

    def patched(*a, **k):
        bir = _json.loads(orig(*a, **k))
        for fn in bir.get("functions", []):
            for bb in fn.get("blocks", []):
                out = []
                for inst in bb.get("instructions", []):
                    si = inst.get("sync_info")
                    if si and si.get("on_wait") and len(si["on_wait"]) > max_waits:
                        waits = si["on_wait"]
                        extra, keep = waits[:-max_waits], waits[-max_waits:]
                        for w in extra:
                            counter[0] += 1
                            out.append({
                                "debug": inst.get("debug", 0),
                                "engine": inst["engine"],
                                "ins": [], "outs": [],
                                "name": f"I-waitsplit-{counter[0]}",
                                "opcode": "EventSemaphore",
                                "sync_info": {"on_update": [], "on_wait": [w]},
                            })
                        si["on_wait"] = keep
                    out.append(inst)
                bb["instructions"] = out
        return _json.dumps(bir).encode()

    nc.to_json_bytes = patched


# ----------------------------------------------------------------------------
# Block plan: per (sk chunk i, sq 512-chunk j) either None (fully masked ->
# skip) or (a, mask_jqs): a = 128-aligned start column (within the 512 block)
# of the needed sq range; mask_jqs = 128-wide subwindows that need the
# additive mask matmul. Computed from the union of all batches' masks so one
# SPMD program is valid for every core; per-core mask DATA handles the rest.
# ----------------------------------------------------------------------------
def make_plan(mask: np.ndarray):
    need = (mask != 0).any(axis=0)   # [sq, sk]: any batch attends
    allu = (mask != 0).all(axis=0)   # [sq, sk]: unmasked in every batch
    plan = []
    for i in range(NSK):
        row = []
        for j in range(NSJ):
            sub_need = need[512 * j:512 * j + 512, 128 * i:128 * i + 128]
            colneed = sub_need.any(axis=1)  # [512] over sq
            if not colneed.any():
                row.append(None)
                continue
            a = (int(np.argmax(colneed)) // 128) * 128
            mask_jqs = []
            for jq in range(a // 128, 4):
                w = allu[512 * j + 128 * jq:512 * j + 128 * (jq + 1),
                         128 * i:128 * i + 128]
                if not w.all():
                    mask_jqs.append(jq)
            row.append((a, tuple(mask_jqs)))
        plan.append(tuple(row))
    return tuple(plan)


def plan_dense():
    return tuple(tuple((0, (0, 1, 2, 3)) for _ in range(NSJ)) for _ in range(NSK))


# ----------------------------------------------------------------------------
# Device program
# ----------------------------------------------------------------------------
def build_nc(plan, repeat: int = 1):
    nc = bass.Bass("TRN2", target_bir_lowering=False, debug=False)

    qT = nc.declare_dram_parameter("qT", [D, S], BF, isOutput=False)
    kT = nc.declare_dram_parameter("kT", [D, S], BF, isOutput=False)
    vT = nc.declare_dram_parameter("vT", [D, S], BF, isOutput=False)
    wqT = nc.declare_dram_parameter("wqT", [D, C], BF, isOutput=False)
    wkT = nc.declare_dram_parameter("wkT", [D, C], BF, isOutput=False)
    wvT = nc.declare_dram_parameter("wvT", [D, C], BF, isOutput=False)
    woT = nc.declare_dram_parameter("woT", [C, D], BF, isOutput=False)
    bqv = nc.declare_dram_parameter("bqv", [1, C], BF, isOutput=False)
    bkv = nc.declare_dram_parameter("bkv", [1, C], BF, isOutput=False)
    bvv = nc.declare_dram_parameter("bvv", [1, C], BF, isOutput=False)
    maskaT = nc.declare_dram_parameter("maskaT", [S, S], BF, isOutput=False)
    ident = nc.declare_dram_parameter("ident", [P, P], BF, isOutput=False)
    onesr = nc.declare_dram_parameter("onesr", [1, 512], BF, isOutput=False)
    yT = nc.declare_dram_parameter("yT", [D, S], F32, isOutput=True)

    with PatchedTileContext(nc) as tc:
        with (
            tc.tile_pool(name="wpool", bufs=1) as wpool,
            tc.tile_pool(name="xpool", bufs=9) as xpool,
            tc.tile_pool(name="hpool", bufs=1) as hpool,
            tc.tile_pool(name="epool", bufs=2) as epool,
            tc.tile_pool(name="spool", bufs=2) as spool,
            tc.tile_pool(name="ppool", bufs=1, space="PSUM") as ppool,
        ):
            # resident weights / constants
            wq_sb = wpool.tile([P, 8, C], BF, tag="wq")
            wk_sb = wpool.tile([P, 8, C], BF, tag="wk")
            wv_sb = wpool.tile([P, 8, C], BF, tag="wv")
            wo_sb = wpool.tile([P, 4, D], BF, tag="wo")
            mk_sb = wpool.tile([P, NSK, S], BF, tag="mk")
            id_sb = wpool.tile([P, P], BF, tag="id")
            on_sb = wpool.tile([1, 512], BF, tag="on")
            bq_sb = wpool.tile([1, C], BF, tag="bq")
            bk_sb = wpool.tile([1, C], BF, tag="bk")
            bv_sb = wpool.tile([1, C], BF, tag="bv")
            nc.sync.dma_start(wq_sb[:], wqT.rearrange("(o p) c -> p o c", p=P))
            nc.sync.dma_start(wk_sb[:], wkT.rearrange("(o p) c -> p o c", p=P))
            nc.sync.dma_start(wv_sb[:], wvT.rearrange("(o p) c -> p o c", p=P))
            nc.sync.dma_start(wo_sb[:], woT.rearrange("(o p) c -> p o c", p=P))
            nc.sync.dma_start(mk_sb[:], maskaT.rearrange("(o p) c -> p o c", p=P))
            nc.sync.dma_start(id_sb[:], ident[:])
            nc.sync.dma_start(on_sb[:], onesr[:])
            nc.sync.dma_start(bq_sb[:], bqv[:])
            nc.sync.dma_start(bk_sb[:], bkv[:])
            nc.sync.dma_start(bv_sb[:], bvv[:])
            onesf = wpool.tile([P, 64], F32, tag="onesf")
            nc.vector.memset(onesf[:], 1.0)

            def body(it):
                # ---------------- projections ----------------
                qh_sb = hpool.tile([P, 4, S], BF, tag="qh")
                kh_sb = hpool.tile([P, 4, S], BF, tag="kh")
                vh_sb = hpool.tile([P, NSK, 8 * 65], BF, tag="vh")

                # ones columns of vh (softmax denominator)
                vh4 = vh_sb[:, :, :].rearrange("p s (g c) -> p s g c", c=130)
                nc.vector.memset(vh4[:, :, :, 64:65], 1.0)
                nc.vector.memset(vh4[:, :, :, 129:130], 1.0)

                # q/k projections: psum [c:128, s:512], contract d
                for (xdram, w_sb, b_sb, out_sb) in (
                    (qT, wq_sb, bq_sb, qh_sb),
                    (kT, wk_sb, bk_sb, kh_sb),
                ):
                    xt = []
                    for dchunk in range(8):
                        t = xpool.tile([P, S], BF, tag="xt")
                        nc.sync.dma_start(
                            t[:], xdram[P * dchunk:P * (dchunk + 1), :])
                        xt.append(t)
                    for cc in range(4):
                        for sj in range(2):
                            ps = ppool.tile([P, 512], F32, tag="pp", bufs=1)
                            for dchunk in range(8):
                                nc.tensor.matmul(
                                    ps[:],
                                    w_sb[:, dchunk, 128 * cc:128 * (cc + 1)],
                                    xt[dchunk][:, 512 * sj:512 * (sj + 1)],
                                    start=(dchunk == 0), stop=False)
                            nc.tensor.matmul(
                                ps[:], b_sb[0:1, 128 * cc:128 * (cc + 1)],
                                on_sb[0:1, :], start=False, stop=True)
                            nc.scalar.copy(
                                out=out_sb[:, cc, 512 * sj:512 * (sj + 1)],
                                in_=ps[:])
                # v projection: psum [s:128, c:512]
                vt = []
                for dchunk in range(8):
                    t = xpool.tile([P, S], BF, tag="xt")
                    nc.sync.dma_start(t[:], vT[P * dchunk:P * (dchunk + 1), :])
                    vt.append(t)
                for si in range(NSK):
                    ps = ppool.tile([P, 512], F32, tag="pp", bufs=1)
                    for dchunk in range(8):
                        nc.tensor.matmul(
                            ps[:],
                            vt[dchunk][:, 128 * si:128 * (si + 1)],
                            wv_sb[:, dchunk, :],
                            start=(dchunk == 0), stop=False)
                    nc.tensor.matmul(
                        ps[:], on_sb[0:1, 0:128], bv_sb[0:1, :],
                        start=False, stop=True)
                    ps_re = ps[:, :].rearrange("p (g c) -> p g c", c=128)
                    vh_re = vh_sb[:, si, :].rearrange("p (g c) -> p g c", c=130)
                    nc.scalar.copy(out=vh_re[:, :, 0:64], in_=ps_re[:, :, 0:64])
                    nc.scalar.copy(out=vh_re[:, :, 65:129], in_=ps_re[:, :, 64:128])

                # ---------------- attention ----------------
                attn_sb = hpool.tile([P, 4, S], BF, tag="attn")
                for g in range(4):          # head pair (2g, 2g+1)
                    ex = epool.tile([P, 2, NSK, S], BF, tag="ex")
                    for i in range(NSK):
                        for j in range(NSJ):
                            pl = plan[i][j]
                            if pl is None:
                                continue
                            a, mask_jqs = pl
                            n = 512 - a
                            for half in range(2):
                                p0 = 64 * half
                                ps = ppool.tile(
                                    [P, 512], F32, tag=f"sc{half}{j}", bufs=1)
                                nc.tensor.matmul(
                                    ps[:, a:512],
                                    kh_sb[p0:p0 + 64, g, 128 * i:128 * (i + 1)],
                                    qh_sb[p0:p0 + 64, g, 512 * j + a:512 * (j + 1)],
                                    start=True, stop=(not mask_jqs))
                                for nq, jq in enumerate(mask_jqs):
                                    nc.tensor.matmul(
                                        ps[:, 128 * jq:128 * (jq + 1)],
                                        id_sb[:],
                                        mk_sb[:, i, 512 * j + 128 * jq:
                                              512 * j + 128 * (jq + 1)],
                                        start=False,
                                        stop=(nq == len(mask_jqs) - 1))
                                nc.scalar.activation(
                                    ex[:, half, i, 512 * j + a:512 * (j + 1)],
                                    ps[:, a:512],
                                    mybir.ActivationFunctionType.Exp)
                    for half in range(2):
                        h = 2 * g + half
                        for j in range(NSJ):
                            incl = [i for i in range(NSK) if plan[i][j] is not None]
                            if not incl:
                                continue
                            av = ppool.tile([65, 512], F32, tag="av", bufs=2)
                            for n_i, i in enumerate(incl):
                                a, _ = plan[i][j]
                                nc.tensor.matmul(
                                    av[:, a:512],
                                    vh_sb[:, i, 65 * h:65 * h + 65],
                                    ex[:, half, i, 512 * j + a:512 * (j + 1)],
                                    start=(n_i == 0), stop=(n_i == len(incl) - 1))
                            # normalize rows 0..63 by row 64, write attnT:
                            # reciprocal of the denominator row, broadcast
                            # across 64 partitions via a K=1 fp32 matmul
                            # (ones column at base partition 64), then a
                            # DVE multiply.
                            rc = spool.tile([P, 512], F32, tag="rc")
                            nc.vector.reciprocal(
                                out=rc[64:65, :], in_=av[64:65, :])
                            rb = ppool.tile([64, 512], F32, tag="rb", bufs=1)
                            nc.tensor.matmul(
                                rb[:], onesf[64:65, 0:64], rc[64:65, :],
                                start=True, stop=True)
                            rbs = spool.tile([64, 512], F32, tag="rbs")
                            nc.scalar.copy(out=rbs[:], in_=rb[:])
                            if half == 0:
                                nc.vector.tensor_tensor(
                                    attn_sb[0:64, g, 512 * j:512 * (j + 1)],
                                    av[0:64, :], rbs[:],
                                    mybir.AluOpType.mult)
                            else:
                                st = spool.tile([64, 512], BF, tag="st")
                                nc.vector.tensor_tensor(
                                    st[:], av[0:64, :], rbs[:],
                                    mybir.AluOpType.mult)
                                nc.sync.dma_start(
                                    attn_sb[64:128, g, 512 * j:512 * (j + 1)],
                                    st[:])

                # ---------------- output projection ----------------
                for m in range(8):
                    for j in range(NSJ):
                        ps = ppool.tile([P, 512], F32, tag="pp", bufs=1)
                        for cc in range(4):
                            nc.tensor.matmul(
                                ps[:],
                                wo_sb[:, cc, 128 * m:128 * (m + 1)],
                                attn_sb[:, cc, 512 * j:512 * (j + 1)],
                                start=(cc == 0), stop=(cc == 3))
                        ys = spool.tile([P, 512], F32, tag="ys")
                        nc.vector.tensor_copy(ys[:], ps[:])
                        nc.sync.dma_start(
                            yT[128 * m:128 * (m + 1), 512 * j:512 * (j + 1)],
                            ys[:])

            if repeat == 1:
                body(0)
            else:
                with tc.For_i(0, repeat, 1):
                    body(0)
    _install_wait_split(nc)
    return nc


# ----------------------------------------------------------------------------
# Host-side shard / unshard + persistent jitted runner
# ----------------------------------------------------------------------------
def shard_inputs(q, k, v, mask, Wq, bq, Wk, bk, Wv, bv, Wo, bo):
    q = np.asarray(q, np.float32)
    k = np.asarray(k, np.float32)
    v = np.asarray(v, np.float32)
    mask = np.asarray(mask)
    Wq = np.asarray(Wq, np.float32); bq = np.asarray(bq, np.float32)
    Wk = np.asarray(Wk, np.float32); bk = np.asarray(bk, np.float32)
    Wv = np.asarray(Wv, np.float32); bv = np.asarray(bv, np.float32)
    Wo = np.asarray(Wo, np.float32)
    sc = 1.0 / math.sqrt(DK)
    ident = np.eye(P, dtype=BF16)
    onesr = np.ones((1, 512), BF16)
    in_maps = []
    for c in range(8):
        b = c // 2
        ch = slice(C * (c % 2), C * (c % 2) + C)
        maskaT = np.where(mask[b].T == 0, np.float32(NEG), np.float32(0.0))
        in_maps.append({
            "qT": np.ascontiguousarray(q[b].T).astype(BF16),
            "kT": np.ascontiguousarray(k[b].T).astype(BF16),
            "vT": np.ascontiguousarray(v[b].T).astype(BF16),
            "wqT": np.ascontiguousarray((Wq[ch, :] * sc).T).astype(BF16),
            "wkT": np.ascontiguousarray(Wk[ch, :].T).astype(BF16),
            "wvT": np.ascontiguousarray(Wv[ch, :].T).astype(BF16),
            "woT": np.ascontiguousarray(Wo[:, ch].T).astype(BF16),
            "bqv": (bq[ch] * sc).reshape(1, C).astype(BF16),
            "bkv": bk[ch].reshape(1, C).astype(BF16),
            "bvv": bv[ch].reshape(1, C).astype(BF16),
            "maskaT": np.ascontiguousarray(maskaT).astype(BF16),
            "ident": ident,
            "onesr": onesr,
        })
    return in_maps


def gather_output(results, bo):
    bo = np.asarray(bo, np.float32)
    y = np.empty((B, S, D), np.float32)
    for b in range(B):
        yt = results[2 * b]["yT"] + results[2 * b + 1]["yT"]
        y[b] = yt.T + bo[None, :]
    return y


class BassRunner:
    """jit-cached shard_map execution of a bass SPMD program on 8 cores."""

    def __init__(self, nc, n_cores=8):
        import jax
        from jax.sharding import Mesh, PartitionSpec
        from jax.experimental.shard_map import shard_map
        from concourse.bass2jax import (
            _bass_exec_p, install_neuronx_cc_hook, partition_id_tensor)

        install_neuronx_cc_hook()
        self.jax = jax
        self.nc = nc
        self.n_cores = n_cores
        partition_name = (nc.partition_id_tensor.name
                          if nc.partition_id_tensor else None)
        in_names, out_names, out_avals, zero_outs = [], [], [], []
        for alloc in nc.m.functions[0].allocations:
            if not isinstance(alloc, mybir.MemoryLocationSet):
                continue
            name = alloc.memorylocations[0].name
            if alloc.kind == "ExternalInput":
                if name != partition_name:
                    in_names.append(name)
            elif alloc.kind == "ExternalOutput":
                out_names.append(name)
                shape = tuple(alloc.tensor_shape)
                dtype = mybir.dt.np(alloc.dtype)
                out_avals.append(jax.core.ShapedArray(shape, dtype))
                zero_outs.append(np.zeros(shape, dtype))
        self.in_names = in_names
        self.out_names = out_names
        self.zero_outs = zero_outs
        n_params = len(in_names)
        self.n_params = n_params
        all_in = in_names + out_names + ([partition_name] if partition_name else [])
        donate = tuple(range(n_params, n_params + len(out_names)))

        def _body(*args):
            operands = list(args)
            if partition_name is not None:
                operands.append(partition_id_tensor())
            return tuple(_bass_exec_p.bind(
                *operands, out_avals=tuple(out_avals), in_names=tuple(all_in),
                out_names=tuple(out_names), lowering_input_output_aliases=(),
                sim_require_finite=False, sim_require_nnan=False, nc=nc))

        devices = jax.devices()[:n_cores]
        mesh = Mesh(np.asarray(devices), ("core",))
        in_specs = (PartitionSpec("core"),) * (n_params + len(out_names))
        out_specs = (PartitionSpec("core"),) * len(out_names)
        self.sharded = jax.jit(
            shard_map(_body, mesh=mesh, in_specs=in_specs,
                      out_specs=out_specs, check_rep=False),
            donate_argnums=donate, keep_unused=True)

    def concat_inputs(self, in_maps):
        per_core = [[np.asarray(m[n]) for n in self.in_names] for m in in_maps]
        concat_in = [np.concatenate([per_core[c][i] for c in range(self.n_cores)],
                                    axis=0) for i in range(self.n_params)]
        concat_zero = [np.concatenate([z] * self.n_cores, axis=0)
                       for z in self.zero_outs]
        return concat_in, concat_zero

    def run(self, in_maps):
        concat_in, concat_zero = self.concat_inputs(in_maps)
        outs = [np.asarray(o) for o in self.sharded(*concat_in, *concat_zero)]
        results = []
        for c in range(self.n_cores):
            res = {}
            for i, name in enumerate(self.out_names):
                rows = outs[i].shape[0] // self.n_cores
                res[name] = outs[i][c * rows:(c + 1) * rows]
            results.append(res)
        return results


_RUNNERS = {}


def _get_runner(plan):
    key = plan
    if key not in _RUNNERS:
        _RUNNERS[key] = BassRunner(build_nc(plan, repeat=1))
    return _RUNNERS[key]


def kernel(q, k, v, mask, Wq, bq, Wk, bk, Wv, bv, Wo, bo):
    mask = np.asarray(mask)
    plan = make_plan(mask)
    in_maps = shard_inputs(q, k, v, mask, Wq, bq, Wk, bk, Wv, bv, Wo, bo)
    runner = _get_runner(plan)
    results = runner.run(in_maps)
    return gather_output(results, bo)



# revision 1
# speedup vs baseline: 1.5421x; 1.5421x over previous
"""Multi-head attention (B=4, S=1024, D=1024, H=16) on 8 Trainium2 NeuronCores.

Sharding (Megatron-style, per the hint): core c handles batch b = c//2 and
head-group hg = c%2 (8 heads = 512 channels of the QKV projections). Each
core computes its 8 heads' attention plus the partial output projection
y_part = attn_local @ Wo[:, ch].T; the host sums the two partials per batch
and adds bo.

Device kernel (bf16 matmuls, fp32 PSUM):
  - projections produce qhT/khT in [c, s] layout and vh in [s, c] layout
    (with a ones column appended per head for the softmax denominator)
  - scoresT[sk, sq] per head via khT-stationary matmuls, two heads row-packed
    into PE row-groups 0-63/64-127 so they run concurrently
  - additive mask (0 / -1e9, precomputed on host) accumulated into the score
    PSUM via an identity-stationary matmul
  - exp on ScalarE -> bf16 expT tiles; AV via vh-stationary matmuls gives
    attnT[c, sq] with the denominator in row 64; normalization via DVE
    reciprocal of the denominator row, a K=1 fp32 matmul that broadcasts it
    across partitions, and a DVE multiply
  - block plan computed from the actual mask contents skips fully-masked
    [sk-chunk x sq-range] blocks (causal mask -> ~2x attention-phase savings);
    an all-ones mask degenerates to the dense plan, still correct
"""

import math

import numpy as np
import ml_dtypes

import concourse.bass as bass
import concourse.mybir as mybir
import concourse.tile as tile
from bass_rust import ScopedClock, SyncInfo

BF16 = ml_dtypes.bfloat16
F32 = mybir.dt.float32
BF = mybir.dt.bfloat16

P = 128
B, S, D, H = 4, 1024, 1024, 16
DK = D // H           # 64
HLOC = H // 2         # 8 heads per core
C = HLOC * DK         # 512 local channels
NSK = S // P          # 8 sk chunks of 128
NSJ = 2               # sq chunks of 512
NEG = -1.0e9


# ----------------------------------------------------------------------------
# Walrus in this container rejects Drain instructions carrying more than one
# sync-wait command, and the leader/follower all-engine barrier. Override the
# TileContext exit sequence: split the tail drain's waits one-per-Drain and
# use the sem-only (EVSEM) barrier.
# ----------------------------------------------------------------------------
class PatchedTileContext(tile.TileContext):
    def _drain_and_barrier(self, tick_clock, wait_clock):
        nc = self.nc
        probe = nc.sync.drain()
        wait_clock.add_sem_waits(
            probe.ins, ScopedClock({None: tick_clock.global_clock})
        )
        si = probe.ins.sync_info
        waits = list(si.on_wait) if si is not None else []
        if len(waits) > 1:
            probe.ins.sync_info = SyncInfo(on_wait=waits[:1], on_update=[])
            for w in waits[1:]:
                extra = nc.sync.drain()
                extra.ins.sync_info = SyncInfo(on_wait=[w], on_update=[])
        nc.all_engine_barrier(sem_only=True)
        assert self.sems is not None
        popped = nc._tile_sem_poison_stack.pop()
        assert popped is self._sem_poison
        nc.clear_and_free_semaphores(list(self.sems.allocated().values()))
        nc.all_engine_barrier(sem_only=True)


def _install_wait_split(nc, max_waits: int = 1):
    """Walrus in this container rejects instructions carrying more than one
    sync-wait command. Post-process the serialized BIR: hoist excess on_wait
    entries of any instruction onto EventSemaphore instructions inserted just
    before it on the same engine (sequencers execute in order, so this is
    equivalent)."""
    import json as _json

    orig = nc.to_json_bytes
    counter = [0]

    def patched(*a, **k):
        bir = _json.loads(orig(*a, **k))
        for fn in bir.get("functions", []):
            for bb in fn.get("blocks", []):
                out = []
                for inst in bb.get("instructions", []):
                    si = inst.get("sync_info")
                    if si and si.get("on_wait") and len(si["on_wait"]) > max_waits:
                        waits = si["on_wait"]
                        extra, keep = waits[:-max_waits], waits[-max_waits:]
                        for w in extra:
                            counter[0] += 1
                            out.append({
                                "debug": inst.get("debug", 0),
                                "engine": inst["engine"],
                                "ins": [], "outs": [],
                                "name": f"I-waitsplit-{counter[0]}",
                                "opcode": "EventSemaphore",
                                "sync_info": {"on_update": [], "on_wait": [w]},
                            })
                        si["on_wait"] = keep
                    out.append(inst)
                bb["instructions"] = out
        return _json.dumps(bir).encode()

    nc.to_json_bytes = patched


# ----------------------------------------------------------------------------
# Block plan: per (sk chunk i, sq 512-chunk j) either None (fully masked ->
# skip) or (a, mask_jqs): a = 128-aligned start column (within the 512 block)
# of the needed sq range; mask_jqs = 128-wide subwindows that need the
# additive mask matmul. Computed from the union of all batches' masks so one
# SPMD program is valid for every core; per-core mask DATA handles the rest.
# ----------------------------------------------------------------------------
def make_plan(mask: np.ndarray):
    need = (mask != 0).any(axis=0)   # [sq, sk]: any batch attends
    allu = (mask != 0).all(axis=0)   # [sq, sk]: unmasked in every batch
    plan = []
    for i in range(NSK):
        row = []
        for j in range(NSJ):
            sub_need = need[512 * j:512 * j + 512, 128 * i:128 * i + 128]
            colneed = sub_need.any(axis=1)  # [512] over sq
            if not colneed.any():
                row.append(None)
                continue
            a = (int(np.argmax(colneed)) // 128) * 128
            mask_jqs = []
            for jq in range(a // 128, 4):
                w = allu[512 * j + 128 * jq:512 * j + 128 * (jq + 1),
                         128 * i:128 * i + 128]
                if not w.all():
                    mask_jqs.append(jq)
            row.append((a, tuple(mask_jqs)))
        plan.append(tuple(row))
    return tuple(plan)


def plan_dense():
    return tuple(tuple((0, (0, 1, 2, 3)) for _ in range(NSJ)) for _ in range(NSK))


# ----------------------------------------------------------------------------
# Device program
# ----------------------------------------------------------------------------
def build_nc(plan, repeat: int = 1):
    nc = bass.Bass("TRN2", target_bir_lowering=False, debug=False)

    qT = nc.declare_dram_parameter("qT", [D, S], BF, isOutput=False)
    kT = nc.declare_dram_parameter("kT", [D, S], BF, isOutput=False)
    vT = nc.declare_dram_parameter("vT", [D, S], BF, isOutput=False)
    wqT = nc.declare_dram_parameter("wqT", [D, C], BF, isOutput=False)
    wkT = nc.declare_dram_parameter("wkT", [D, C], BF, isOutput=False)
    wvT = nc.declare_dram_parameter("wvT", [D, C], BF, isOutput=False)
    woT = nc.declare_dram_parameter("woT", [C, D], BF, isOutput=False)
    bqv = nc.declare_dram_parameter("bqv", [1, C], BF, isOutput=False)
    bkv = nc.declare_dram_parameter("bkv", [1, C], BF, isOutput=False)
    bvv = nc.declare_dram_parameter("bvv", [1, C], BF, isOutput=False)
    maskaT = nc.declare_dram_parameter("maskaT", [S, S], BF, isOutput=False)
    ident = nc.declare_dram_parameter("ident", [P, P], BF, isOutput=False)
    onesr = nc.declare_dram_parameter("onesr", [1, 512], BF, isOutput=False)
    yT = nc.declare_dram_parameter("yT", [D, S], F32, isOutput=True)

    with PatchedTileContext(nc) as tc:
        with (
            tc.tile_pool(name="wpool", bufs=1) as wpool,
            tc.tile_pool(name="xpool", bufs=9) as xpool,
            tc.tile_pool(name="hpool", bufs=1) as hpool,
            tc.tile_pool(name="epool", bufs=2) as epool,
            tc.tile_pool(name="spool", bufs=2) as spool,
            tc.tile_pool(name="ppool", bufs=1, space="PSUM") as ppool,
        ):
            # resident weights / constants
            wq_sb = wpool.tile([P, 8, C], BF, tag="wq")
            wk_sb = wpool.tile([P, 8, C], BF, tag="wk")
            wv_sb = wpool.tile([P, 8, C], BF, tag="wv")
            wo_sb = wpool.tile([P, 4, D], BF, tag="wo")
            mk_sb = wpool.tile([P, NSK, S], BF, tag="mk")
            id_sb = wpool.tile([P, P], BF, tag="id")
            on_sb = wpool.tile([1, 512], BF, tag="on")
            bq_sb = wpool.tile([1, C], BF, tag="bq")
            bk_sb = wpool.tile([1, C], BF, tag="bk")
            bv_sb = wpool.tile([1, C], BF, tag="bv")
            nc.sync.dma_start(wq_sb[:], wqT.rearrange("(o p) c -> p o c", p=P))
            nc.sync.dma_start(wk_sb[:], wkT.rearrange("(o p) c -> p o c", p=P))
            nc.sync.dma_start(wv_sb[:], wvT.rearrange("(o p) c -> p o c", p=P))
            nc.sync.dma_start(wo_sb[:], woT.rearrange("(o p) c -> p o c", p=P))
            nc.sync.dma_start(mk_sb[:], maskaT.rearrange("(o p) c -> p o c", p=P))
            nc.sync.dma_start(id_sb[:], ident[:])
            nc.sync.dma_start(on_sb[:], onesr[:])
            nc.sync.dma_start(bq_sb[:], bqv[:])
            nc.sync.dma_start(bk_sb[:], bkv[:])
            nc.sync.dma_start(bv_sb[:], bvv[:])
            onesf = wpool.tile([P, 64], F32, tag="onesf")
            nc.vector.memset(onesf[:], 1.0)

            def body(it):
                # ---------------- projections ----------------
                qh_sb = hpool.tile([P, 4, S], BF, tag="qh")
                kh_sb = hpool.tile([P, 4, S], BF, tag="kh")
                vh_sb = hpool.tile([P, NSK, 8 * 65], BF, tag="vh")

                # ones columns of vh (softmax denominator)
                vh4 = vh_sb[:, :, :].rearrange("p s (g c) -> p s g c", c=130)
                nc.vector.memset(vh4[:, :, :, 64:65], 1.0)
                nc.vector.memset(vh4[:, :, :, 129:130], 1.0)

                # q/k projections: psum [c:128, s:512], contract d
                for (xdram, w_sb, b_sb, out_sb) in (
                    (qT, wq_sb, bq_sb, qh_sb),
                    (kT, wk_sb, bk_sb, kh_sb),
                ):
                    xt = []
                    for dchunk in range(8):
                        t = xpool.tile([P, S], BF, tag="xt")
                        nc.sync.dma_start(
                            t[:], xdram[P * dchunk:P * (dchunk + 1), :])
                        xt.append(t)
                    for cc in range(4):
                        for sj in range(2):
                            ps = ppool.tile([P, 512], F32, tag="pp", bufs=1)
                            for dchunk in range(8):
                                nc.tensor.matmul(
                                    ps[:],
                                    w_sb[:, dchunk, 128 * cc:128 * (cc + 1)],
                                    xt[dchunk][:, 512 * sj:512 * (sj + 1)],
                                    start=(dchunk == 0), stop=False)
                            nc.tensor.matmul(
                                ps[:], b_sb[0:1, 128 * cc:128 * (cc + 1)],
                                on_sb[0:1, :], start=False, stop=True)
                            nc.scalar.copy(
                                out=out_sb[:, cc, 512 * sj:512 * (sj + 1)],
                                in_=ps[:])
                # v projection: psum [s:128, c:512]
                vt = []
                for dchunk in range(8):
                    t = xpool.tile([P, S], BF, tag="xt")
                    nc.sync.dma_start(t[:], vT[P * dchunk:P * (dchunk + 1), :])
                    vt.append(t)
                for si in range(NSK):
                    ps = ppool.tile([P, 512], F32, tag="pp", bufs=1)
                    for dchunk in range(8):
                        nc.tensor.matmul(
                            ps[:],
                            vt[dchunk][:, 128 * si:128 * (si + 1)],
                            wv_sb[:, dchunk, :],
                            start=(dchunk == 0), stop=False)
                    nc.tensor.matmul(
                        ps[:], on_sb[0:1, 0:128], bv_sb[0:1, :],
                        start=False, stop=True)
                    ps_re = ps[:, :].rearrange("p (g c) -> p g c", c=128)
                    vh_re = vh_sb[:, si, :].rearrange("p (g c) -> p g c", c=130)
                    nc.scalar.copy(out=vh_re[:, :, 0:64], in_=ps_re[:, :, 0:64])
                    nc.scalar.copy(out=vh_re[:, :, 65:129], in_=ps_re[:, :, 64:128])

                # ---------------- attention ----------------
                attn_sb = hpool.tile([P, 4, S], BF, tag="attn")
                for g in range(4):          # head pair (2g, 2g+1)
                    ex = epool.tile([P, 2, NSK, S], BF, tag="ex")
                    for i in range(NSK):
                        for j in range(NSJ):
                            pl = plan[i][j]
                            if pl is None:
                                continue
                            a, mask_jqs = pl
                            n = 512 - a
                            for half in range(2):
                                p0 = 64 * half
                                ps = ppool.tile(
                                    [P, 512], F32, tag=f"sc{half}{j}", bufs=1)
                                nc.tensor.matmul(
                                    ps[:, a:512],
                                    kh_sb[p0:p0 + 64, g, 128 * i:128 * (i + 1)],
                                    qh_sb[p0:p0 + 64, g, 512 * j + a:512 * (j + 1)],
                                    start=True, stop=(not mask_jqs))
                                for nq, jq in enumerate(mask_jqs):
                                    nc.tensor.matmul(
                                        ps[:, 128 * jq:128 * (jq + 1)],
                                        id_sb[:],
                                        mk_sb[:, i, 512 * j + 128 * jq:
                                              512 * j + 128 * (jq + 1)],
                                        start=False,
                                        stop=(nq == len(mask_jqs) - 1))
                                nc.scalar.activation(
                                    ex[:, half, i, 512 * j + a:512 * (j + 1)],
                                    ps[:, a:512],
                                    mybir.ActivationFunctionType.Exp)
                    for half in range(2):
                        h = 2 * g + half
                        for j in range(NSJ):
                            incl = [i for i in range(NSK) if plan[i][j] is not None]
                            if not incl:
                                continue
                            av = ppool.tile([65, 512], F32, tag="av", bufs=2)
                            for n_i, i in enumerate(incl):
                                a, _ = plan[i][j]
                                nc.tensor.matmul(
                                    av[:, a:512],
                                    vh_sb[:, i, 65 * h:65 * h + 65],
                                    ex[:, half, i, 512 * j + a:512 * (j + 1)],
                                    start=(n_i == 0), stop=(n_i == len(incl) - 1))
                            # normalize rows 0..63 by row 64, write attnT:
                            # reciprocal of the denominator row, broadcast
                            # across 64 partitions via a K=1 fp32 matmul
                            # (ones column at base partition 64), then a
                            # DVE multiply.
                            rc = spool.tile([P, 512], F32, tag="rc")
                            nc.vector.reciprocal(
                                out=rc[64:65, :], in_=av[64:65, :])
                            rb = ppool.tile([64, 512], F32, tag="rb", bufs=1)
                            nc.tensor.matmul(
                                rb[:], onesf[64:65, 0:64], rc[64:65, :],
                                start=True, stop=True)
                            rbs = spool.tile([64, 512], F32, tag="rbs")
                            nc.scalar.copy(out=rbs[:], in_=rb[:])
                            if half == 0:
                                nc.vector.tensor_tensor(
                                    attn_sb[0:64, g, 512 * j:512 * (j + 1)],
                                    av[0:64, :], rbs[:],
                                    mybir.AluOpType.mult)
                            else:
                                st = spool.tile([64, 512], BF, tag="st")
                                nc.vector.tensor_tensor(
                                    st[:], av[0:64, :], rbs[:],
                                    mybir.AluOpType.mult)
                                nc.sync.dma_start(
                                    attn_sb[64:128, g, 512 * j:512 * (j + 1)],
                                    st[:])

                # ---------------- output projection ----------------
                for m in range(8):
                    for j in range(NSJ):
                        ps = ppool.tile([P, 512], F32, tag="pp", bufs=1)
                        for cc in range(4):
                            nc.tensor.matmul(
                                ps[:],
                                wo_sb[:, cc, 128 * m:128 * (m + 1)],
                                attn_sb[:, cc, 512 * j:512 * (j + 1)],
                                start=(cc == 0), stop=(cc == 3))
                        ys = spool.tile([P, 512], F32, tag="ys")
                        nc.vector.tensor_copy(ys[:], ps[:])
                        nc.sync.dma_start(
                            yT[128 * m:128 * (m + 1), 512 * j:512 * (j + 1)],
                            ys[:])

            if repeat == 1:
                body(0)
            else:
                with tc.For_i(0, repeat, 1):
                    body(0)
    _install_wait_split(nc)
    return nc


# ----------------------------------------------------------------------------
# Host-side shard / unshard + persistent jitted runner
# ----------------------------------------------------------------------------
def shard_inputs(q, k, v, mask, Wq, bq, Wk, bk, Wv, bv, Wo, bo):
    q = np.asarray(q, np.float32)
    k = np.asarray(k, np.float32)
    v = np.asarray(v, np.float32)
    mask = np.asarray(mask)
    Wq = np.asarray(Wq, np.float32); bq = np.asarray(bq, np.float32)
    Wk = np.asarray(Wk, np.float32); bk = np.asarray(bk, np.float32)
    Wv = np.asarray(Wv, np.float32); bv = np.asarray(bv, np.float32)
    Wo = np.asarray(Wo, np.float32)
    sc = 1.0 / math.sqrt(DK)
    ident = np.eye(P, dtype=BF16)
    onesr = np.ones((1, 512), BF16)
    in_maps = []
    for c in range(8):
        b = c // 2
        ch = slice(C * (c % 2), C * (c % 2) + C)
        maskaT = np.where(mask[b].T == 0, np.float32(NEG), np.float32(0.0))
        in_maps.append({
            "qT": np.ascontiguousarray(q[b].T).astype(BF16),
            "kT": np.ascontiguousarray(k[b].T).astype(BF16),
            "vT": np.ascontiguousarray(v[b].T).astype(BF16),
            "wqT": np.ascontiguousarray((Wq[ch, :] * sc).T).astype(BF16),
            "wkT": np.ascontiguousarray(Wk[ch, :].T).astype(BF16),
            "wvT": np.ascontiguousarray(Wv[ch, :].T).astype(BF16),
            "woT": np.ascontiguousarray(Wo[:, ch].T).astype(BF16),
            "bqv": (bq[ch] * sc).reshape(1, C).astype(BF16),
            "bkv": bk[ch].reshape(1, C).astype(BF16),
            "bvv": bv[ch].reshape(1, C).astype(BF16),
            "maskaT": np.ascontiguousarray(maskaT).astype(BF16),
            "ident": ident,
            "onesr": onesr,
        })
    return in_maps


def gather_output(results, bo):
    bo = np.asarray(bo, np.float32)
    y = np.empty((B, S, D), np.float32)
    for b in range(B):
        yt = results[2 * b]["yT"] + results[2 * b + 1]["yT"]
        y[b] = yt.T + bo[None, :]
    return y


class BassRunner:
    """jit-cached shard_map execution of a bass SPMD program on 8 cores."""

    def __init__(self, nc, n_cores=8):
        import jax
        from jax.sharding import Mesh, PartitionSpec
        from jax.experimental.shard_map import shard_map
        from concourse.bass2jax import (
            _bass_exec_p, install_neuronx_cc_hook, partition_id_tensor)

        install_neuronx_cc_hook()
        self.jax = jax
        self.nc = nc
        self.n_cores = n_cores
        partition_name = (nc.partition_id_tensor.name
                          if nc.partition_id_tensor else None)
        in_names, out_names, out_avals, zero_outs = [], [], [], []
        for alloc in nc.m.functions[0].allocations:
            if not isinstance(alloc, mybir.MemoryLocationSet):
                continue
            name = alloc.memorylocations[0].name
            if alloc.kind == "ExternalInput":
                if name != partition_name:
                    in_names.append(name)
            elif alloc.kind == "ExternalOutput":
                out_names.append(name)
                shape = tuple(alloc.tensor_shape)
                dtype = mybir.dt.np(alloc.dtype)
                out_avals.append(jax.core.ShapedArray(shape, dtype))
                zero_outs.append(np.zeros(shape, dtype))
        self.in_names = in_names
        self.out_names = out_names
        self.zero_outs = zero_outs
        n_params = len(in_names)
        self.n_params = n_params
        all_in = in_names + out_names + ([partition_name] if partition_name else [])
        donate = tuple(range(n_params, n_params + len(out_names)))

        def _body(*args):
            operands = list(args)
            if partition_name is not None:
                operands.append(partition_id_tensor())
            return tuple(_bass_exec_p.bind(
                *operands, out_avals=tuple(out_avals), in_names=tuple(all_in),
                out_names=tuple(out_names), lowering_input_output_aliases=(),
                sim_require_finite=False, sim_require_nnan=False, nc=nc))

        devices = jax.devices()[:n_cores]
        mesh = Mesh(np.asarray(devices), ("core",))
        in_specs = (PartitionSpec("core"),) * (n_params + len(out_names))
        out_specs = (PartitionSpec("core"),) * len(out_names)
        self.sharded = jax.jit(
            shard_map(_body, mesh=mesh, in_specs=in_specs,
                      out_specs=out_specs, check_rep=False),
            donate_argnums=donate, keep_unused=True)

    def concat_inputs(self, in_maps):
        per_core = [[np.asarray(m[n]) for n in self.in_names] for m in in_maps]
        concat_in = [np.concatenate([per_core[c][i] for c in range(self.n_cores)],
                                    axis=0) for i in range(self.n_params)]
        concat_zero = [np.concatenate([z] * self.n_cores, axis=0)
                       for z in self.zero_outs]
        return concat_in, concat_zero

    def run(self, in_maps):
        concat_in, concat_zero = self.concat_inputs(in_maps)
        outs = [np.asarray(o) for o in self.sharded(*concat_in, *concat_zero)]
        results = []
        for c in range(self.n_cores):
            res = {}
            for i, name in enumerate(self.out_names):
                rows = outs[i].shape[0] // self.n_cores
                res[name] = outs[i][c * rows:(c + 1) * rows]
            results.append(res)
        return results


_RUNNERS = {}


def _get_runner(plan):
    key = plan
    if key not in _RUNNERS:
        _RUNNERS[key] = BassRunner(build_nc(plan, repeat=1))
    return _RUNNERS[key]


def kernel(q, k, v, mask, Wq, bq, Wk, bk, Wv, bv, Wo, bo):
    mask = np.asarray(mask)
    plan = make_plan(mask)
    in_maps = shard_inputs(q, k, v, mask, Wq, bq, Wk, bk, Wv, bv, Wo, bo)
    runner = _get_runner(plan)
    results = runner.run(in_maps)
    return gather_output(results, bo)

